# revision 13
# baseline (speedup 1.0000x reference)
"""GQA attention kernel for Trainium2, sharded over 8 NeuronCores.

Sharding: core c = b*4 + g handles batch b and GQA group g (4 query heads
+ 1 KV head). Wq/Wk/Wv column-sharded per group, Wo row-sharded; the host
sums the 4 per-group partial outputs per batch (partials in bf16).

Device layout:
  - x is passed transposed (xT [D, S]) so Q^T/K^T/V^T project directly
    into [head_dim, S] layout (head_dim on partitions); V is then
    transposed back to natural [S, head_dim] via the DMA xbar.
  - Q/K head dims are de-interleaved host-side (even dims then odd dims)
    by permuting Wq/Wk columns; scores are invariant to a shared
    permutation of Q/K dims.  RoPE is 4 DVE ops per [128,512] chunk
    using pre-duplicated cos ([c;c]) and pre-signed sin ([-s;+s]).
  - Attention computes scoresT [key, query]; softmax exp output is
    directly the lhs^T operand for the P@V matmul.  Scores for two
    adjacent key chunks share one [128,2,512] PSUM tile so exp runs at
    free-dim 1024.  Probabilities/denominators in bf16.
  - Causal banding at 256-query granularity: diagonal chunk pairs only
    compute the allowed query range; triangular masks finish the job
    (gpsimd + DVE).
  - Denominator: bf16 chunk accumulation (DVE 2x), ones-matmul partition
    reduce, reciprocal_approx_fast, gpsimd partition_broadcast.
  - All matmuls 16-bit.  Projection and output-projection matmuls are
    emitted interleaved into the attention loop ("fillers") so the PE
    FIFO never idles behind the ACT-bound exp chain and the HAM clock
    stays warm; leftover fillers carry across q-blocks.
  - DMA order at startup: x chunks race the Wq chunks so the first
    projection matmuls start ~3 us in; Wo loads last.
"""

import sys

if "/opt/trn_rl_repo" not in sys.path:
    sys.path.insert(0, "/opt/trn_rl_repo")

import numpy as np
import ml_dtypes

import concourse.bass as bass
import concourse.bacc as bacc
import concourse.tile as tile
from concourse import mybir
from concourse.bass_utils import run_bass_kernel_spmd

B = 2
S = 2048
D = 2048
N_HEADS = 16
N_KV = 4
DH = 128
NH = 4  # query heads per core
N_CORES = 8

INV_SQRT_DH = 1.0 / np.sqrt(DH)
F32 = mybir.dt.float32
BF16 = mybir.dt.bfloat16


def build_program(s=S, d=D):
    """Per-core program: 4 query heads + 1 KV head of causal GQA."""
    kc_n = d // 128       # contraction chunks
    qb_n = s // 512       # q-blocks / s-chunks

    nc = bacc.Bacc("TRN2", target_bir_lowering=False, debug=False,
                   num_devices=N_CORES)
    xT = nc.declare_dram_parameter("xT", [d, s], BF16, isOutput=False)
    wq = nc.declare_dram_parameter("wq", [d, NH * DH], BF16, isOutput=False)
    wkv = nc.declare_dram_parameter("wkv", [d, 2 * DH], BF16, isOutput=False)
    wo = nc.declare_dram_parameter("wo", [NH * DH, d], BF16, isOutput=False)
    cosD = nc.declare_dram_parameter("cosD", [128, s], BF16, isOutput=False)
    sinS = nc.declare_dram_parameter("sinS", [128, s], BF16, isOutput=False)
    maskA = nc.declare_dram_parameter("maskA", [128, 128], BF16, isOutput=False)
    maskB = nc.declare_dram_parameter("maskB", [128, 256], BF16, isOutput=False)
    out_p = nc.declare_dram_parameter("out_p", [s, d], BF16, isOutput=True)

    with tile.TileContext(nc) as tc:
        with (
            tc.tile_pool(name="const", bufs=1) as cpool,
            tc.tile_pool(name="xp", bufs=1) as xpool,
            tc.tile_pool(name="act", bufs=1) as apool,
            tc.tile_pool(name="tmp", bufs=1) as tpool,
            tc.tile_pool(name="psum", bufs=1, space="PSUM") as pp,
        ):
            xv = xT.rearrange("(n p) m -> p n m", p=128)
            wqv = wq.rearrange("(n p) m -> p n m", p=128)

            # ---- startup loads: x(sc0) races wq; heavy/late tensors after
            xt0 = xpool.tile([128, kc_n, 512], BF16, tag="xt", bufs=2,
                             name="xt0")
            wq_sb = cpool.tile([128, kc_n, NH * DH], BF16, tag="wq")
            for j4 in range(0, kc_n, 4):
                nc.sync.dma_start(xt0[:, j4:j4 + 4, :], xv[:, j4:j4 + 4, 0:512])
                nc.sync.dma_start(wq_sb[:, j4:j4 + 4, :], wqv[:, j4:j4 + 4, :])
            cos_sb = cpool.tile([128, s], BF16, tag="cos")
            nc.sync.dma_start(cos_sb[:], cosD[:])
            sin_sb = cpool.tile([128, s], BF16, tag="sin")
            nc.sync.dma_start(sin_sb[:], sinS[:])
            wkv_sb = cpool.tile([128, kc_n, 2 * DH], BF16, tag="wkv")
            nc.sync.dma_start(wkv_sb[:], wkv.rearrange("(n p) m -> p n m", p=128))
            mA = cpool.tile([128, 128], BF16, tag="mA")
            nc.sync.dma_start(mA[:], maskA[:])
            mB = cpool.tile([128, 256], BF16, tag="mB")
            nc.sync.dma_start(mB[:], maskB[:])
            ones_col = cpool.tile([128, 1], BF16, tag="ones_col")
            nc.vector.memset(ones_col[:], 1.0)
            wo_sb = cpool.tile([128, NH, d], BF16, tag="wo")
            nc.sync.dma_start(wo_sb[:], wo.rearrange("(n p) m -> p n m", p=128))

            # ---- persistent activations ----
            ktr = apool.tile([128, s], BF16, tag="ktr")
            qtr = {}   # (h, qb) -> tile
            v_sb = {}  # st -> tile
            otr = {}   # (h, qb) -> tile

            def rope(dst, src_psum, sc_i):
                """dst [128,512] bf16 = rope(src) with de-interleaved halves.

                src rows 0:64 = even dims (a), 64:128 = odd dims (b).
                Stage the psum to bf16 SBUF on ACT, then 4 DVE ops at
                bf16 2x rate.  cos_sb = [c; c], sin_sb = [+s; -s]:
                  t1 = qc * cos_sb ; t2 = swap(qc) * sin_sb ; dst = t1+t2
                (swap realized by partition-shifted reads; the sign lives
                in the sin half the INPUT row came from.)
                """
                c = cos_sb[:, sc_i * 512:(sc_i + 1) * 512]
                sg = sin_sb[:, sc_i * 512:(sc_i + 1) * 512]
                qc = tpool.tile([128, 512], BF16, tag="qc", bufs=2)
                nc.scalar.copy(qc[:], src_psum[:])
                t1 = tpool.tile([128, 512], BF16, tag="t1", bufs=2)
                t2 = tpool.tile([128, 512], BF16, tag="t2", bufs=2)
                nc.vector.tensor_mul(t1[:], qc[:], c)
                nc.vector.tensor_mul(t2[0:64, :], qc[64:128, :], sg[64:128, :])
                nc.vector.tensor_mul(t2[64:128, :], qc[0:64, :], sg[0:64, :])
                nc.vector.tensor_add(dst[:], t1[:], t2[:])

            def gen_proj(sc_i):
                """Projection phase for s-chunk sc_i; yields after each PE op."""
                if sc_i == 0:
                    xt = xt0
                else:
                    xt = xpool.tile([128, kc_n, 512], BF16, tag="xt", bufs=2,
                                    name=f"xt{sc_i}")
                    for j4 in range(0, kc_n, 4):
                        nc.sync.dma_start(
                            xt[:, j4:j4 + 4, :],
                            xv[:, j4:j4 + 4, sc_i * 512:(sc_i + 1) * 512])
                # 6 single-psum groups: q0..q3, k, vT (pp rotation overlaps
                # group i+1's matmuls with rope/copy of group i)
                for hh in range(NH + 2):
                    ps = pp.tile([128, 512], F32, tag="pp", bufs=2,
                                 name=f"pj{sc_i}_{hh}")
                    for kc in range(kc_n):
                        if hh < NH:
                            lhsT = wq_sb[:, kc, hh * DH:(hh + 1) * DH]
                        elif hh == NH:
                            lhsT = wkv_sb[:, kc, 0:DH]
                        else:
                            lhsT = wkv_sb[:, kc, DH:2 * DH]
                        nc.tensor.matmul(ps[:], lhsT, xt[:, kc, :],
                                         start=(kc == 0), stop=(kc == kc_n - 1))
                        yield
                    if hh < NH:
                        qtr[(hh, sc_i)] = apool.tile(
                            [128, 512], BF16, tag=f"qtr{hh}", bufs=2,
                            name=f"qtr{hh}_{sc_i}")
                        rope(qtr[(hh, sc_i)][:], ps, sc_i)
                    elif hh == NH:
                        rope(ktr[:, sc_i * 512:(sc_i + 1) * 512], ps, sc_i)
                    else:
                        # vT [dh, 512] -> copy to SBUF -> xbar-transpose to
                        # natural [s, dh] 128-blocks
                        vt_sb = tpool.tile([128, 512], BF16, tag="vt", bufs=2,
                                           name=f"vt{sc_i}")
                        nc.scalar.copy(vt_sb[:], ps[:])
                        for stl in range(4):
                            st = sc_i * 4 + stl
                            v_sb[st] = apool.tile([128, DH], BF16,
                                                  tag=f"v{st}", name=f"v{st}")
                            nc.sync.dma_start_transpose(
                                v_sb[st][:],
                                vt_sb[:, stl * 128:(stl + 1) * 128])

            def gen_outproj(qb, alt_psum=False):
                """Output projection for q-block qb; yields after each PE op."""
                for stl in range(4):
                    st = 4 * qb + stl
                    for dm in range(4):
                        if alt_psum and (stl * 4 + dm) % 2 == 1:
                            wopt = pp.tile([128, 2, 512], F32, tag="sc",
                                           bufs=2, name=f"wop{st}_{dm}")
                            wop = wopt[:, 0, :]
                        else:
                            wop = pp.tile([128, 512], F32, tag="pp", bufs=2,
                                          name=f"wop{st}_{dm}")[:]
                        for h in range(NH):
                            nc.tensor.matmul(
                                wop,
                                otr[(h, qb)][:, stl * 128:(stl + 1) * 128],
                                wo_sb[:, h, dm * 512:(dm + 1) * 512],
                                start=(h == 0), stop=(h == NH - 1))
                            yield
                        osb = tpool.tile([128, 512], BF16, tag="osb", bufs=4,
                                         name=f"osb{st}_{dm}")
                        if (st + dm) % 2 == 0:
                            nc.vector.tensor_copy(osb[:], wop)
                        else:
                            nc.scalar.copy(osb[:], wop)
                        nc.sync.dma_start(
                            out_p[st * 128:(st + 1) * 128,
                                  dm * 512:(dm + 1) * 512], osb[:])

            fillers = []

            def pull(n):
                for _ in range(n):
                    while fillers:
                        try:
                            next(fillers[0])
                            break
                        except StopIteration:
                            fillers.pop(0)
                    else:
                        return

            def drain(gen):
                for _ in gen:
                    pass

            def attn(qb):
                """Attention for q-block qb (4 heads), pulling PE fillers."""
                npair = 2 * (qb + 1)
                pn = 1 if qb == qb_n - 1 else 2  # stretch fillers on last qb
                deferred = [None]

                def flush_deferred():
                    if deferred[0] is not None:
                        deferred[0]()
                        deferred[0] = None

                for h in range(NH):
                    q_t = qtr[(h, qb)]
                    l_acc = tpool.tile([128, 512], BF16, tag="lacc", bufs=2,
                                       name=f"lacc{h}_{qb}")
                    otp = pp.tile([128, 512], F32, tag="ot", bufs=2,
                                  name=f"otp{h}_{qb}")
                    pts = []
                    # scores + exp + mask + denominator accumulate
                    for j in range(npair):
                        jd = j - 2 * qb  # >=0 on diagonal pairs
                        qo = 256 * jd if jd > 0 else 0
                        scp = pp.tile([128, 2, 512], F32, tag="sc", bufs=2,
                                      name=f"scp{h}_{qb}_{j}")
                        nc.tensor.matmul(scp[:, 0, qo:],
                                         ktr[:, (2 * j) * 128:(2 * j + 1) * 128],
                                         q_t[:, qo:], start=True, stop=True)
                        pull(pn)
                        nc.tensor.matmul(scp[:, 1, qo:],
                                         ktr[:, (2 * j + 1) * 128:(2 * j + 2) * 128],
                                         q_t[:, qo:], start=True, stop=True)
                        pull(pn)
                        pt = tpool.tile([128, 2, 512], BF16, tag="pt", bufs=9,
                                        name=f"pt{h}_{qb}_{j}")
                        pts.append((pt, qo))
                        nc.scalar.activation(
                            pt[:, :, qo:], scp[:, :, qo:],
                            mybir.ActivationFunctionType.Exp,
                            scale=float(INV_SQRT_DH))
                        if jd >= 0:
                            # chunk 2j: triangle in cols [qo, qo+128)
                            nc.vector.tensor_mul(pt[:, 0, qo:qo + 128],
                                                 pt[:, 0, qo:qo + 128], mA[:])
                            # chunk 2j+1: first 128 cols dead + triangle
                            nc.vector.tensor_mul(pt[:, 1, qo:qo + 256],
                                                 pt[:, 1, qo:qo + 256], mB[:])
                        if j == 0:
                            nc.vector.tensor_copy(l_acc[:], pt[:, 0, :])
                        else:
                            nc.vector.tensor_add(l_acc[:, qo:], l_acc[:, qo:],
                                                 pt[:, 0, qo:])
                        nc.vector.tensor_add(l_acc[:, qo:], l_acc[:, qo:],
                                             pt[:, 1, qo:])
                    # previous head's normalize now (its all-reduce is done,
                    # so these DVE ops don't stall the vector FIFO)
                    flush_deferred()
                    # P @ V (dense on PE)
                    for j in range(npair):
                        pt, qo = pts[j]
                        nc.tensor.matmul(otp[:, qo:], v_sb[2 * j][:],
                                         pt[:, 0, qo:], start=(j == 0),
                                         stop=False)
                        nc.tensor.matmul(otp[:, qo:], v_sb[2 * j + 1][:],
                                         pt[:, 1, qo:], start=False,
                                         stop=(j == npair - 1))
                    # denominator all-reduce+broadcast on gpsimd; defer the
                    # reciprocal + normalize to the next head's window
                    lred = tpool.tile([128, 512], F32, tag="lred", bufs=2,
                                      name=f"lred{h}_{qb}")
                    nc.gpsimd.partition_all_reduce(
                        lred[:], l_acc[:], 128, bass.bass_isa.ReduceOp.add)
                    otr[(h, qb)] = apool.tile([128, 512], BF16, tag=f"otr{h}",
                                              bufs=2, name=f"otr{h}_{qb}")

                    def tail(h=h, lred=lred, otp=otp):
                        rlb = tpool.tile([128, 512], F32, tag="rlbs", bufs=2,
                                         name=f"rlbs{h}_{qb}")
                        nc.vector.reciprocal_approx_fast(rlb[:], lred[:])
                        nc.vector.tensor_mul(otr[(h, qb)][:], otp[:], rlb[:])

                    deferred[0] = tail
                    pull(2)
                flush_deferred()

            # ---- main schedule ----
            drain(gen_proj(0))
            for qb in range(qb_n):
                if qb + 1 < qb_n:
                    g = gen_proj(qb + 1)
                    next(g, None)  # prime: emit DMAs early
                    fillers.append(g)
                    attn(qb)
                    # proj(qb+1) must fully drain before attn(qb+1): its
                    # rope outputs feed the next q-block's scores.
                    if fillers and fillers[-1] is g:
                        drain(fillers.pop())
                    else:
                        drain(g)
                    fillers.append(gen_outproj(qb))
                else:
                    attn(qb)
            while fillers:
                drain(fillers.pop(0))
            drain(gen_outproj(qb_n - 1, alt_psum=True))

    nc.compile()
    return nc


_PROGRAM = None


def _get_program():
    global _PROGRAM
    if _PROGRAM is None:
        _PROGRAM = build_program()
    return _PROGRAM


_DEINT = np.concatenate([np.arange(0, DH, 2), np.arange(1, DH, 2)])


def make_in_maps(x, rope_cos, rope_sin, Wq, Wk, Wv, Wo, s=S):
    cosT = rope_cos[:s].T.astype(np.float32)   # [64, s]
    sinT = rope_sin[:s].T.astype(np.float32)
    cosD = np.concatenate([cosT, cosT], axis=0).astype(ml_dtypes.bfloat16)
    sinS = np.concatenate([sinT, -sinT], axis=0).astype(ml_dtypes.bfloat16)
    p = np.arange(128)[:, None]
    maskA = (np.arange(128)[None, :] >= p).astype(ml_dtypes.bfloat16)
    maskB = (np.arange(256)[None, :] >= p + 128).astype(ml_dtypes.bfloat16)
    in_maps = []
    for c in range(N_CORES):
        b, g = divmod(c, 4)
        xTc = np.ascontiguousarray(x[b].T.astype(ml_dtypes.bfloat16))
        wq_cols = [
            Wq[:, (g * NH + j) * DH:(g * NH + j + 1) * DH][:, _DEINT]
            for j in range(NH)
        ]
        wq_c = np.ascontiguousarray(
            np.concatenate(wq_cols, axis=1).astype(ml_dtypes.bfloat16))
        wk_c = Wk[:, g * DH:(g + 1) * DH][:, _DEINT]
        wv_c = Wv[:, g * DH:(g + 1) * DH]
        wkv_c = np.ascontiguousarray(
            np.concatenate([wk_c, wv_c], axis=1).astype(ml_dtypes.bfloat16))
        wo_c = np.ascontiguousarray(
            Wo[g * NH * DH:(g + 1) * NH * DH, :].astype(ml_dtypes.bfloat16))
        in_maps.append({
            "xT": xTc, "wq": wq_c, "wkv": wkv_c, "wo": wo_c,
            "cosD": np.ascontiguousarray(cosD),
            "sinS": np.ascontiguousarray(sinS),
            "maskA": maskA, "maskB": maskB,
        })
    return in_maps


def kernel(x, rope_cos, rope_sin, Wq, Wk, Wv, Wo):
    nc = _get_program()
    in_maps = make_in_maps(x, rope_cos, rope_sin, Wq, Wk, Wv, Wo)
    res = run_bass_kernel_spmd(nc, in_maps, list(range(N_CORES)))
    out = np.zeros((B, S, D), dtype=np.float32)
    for c in range(N_CORES):
        b, g = divmod(c, 4)
        out[b] += np.asarray(res.results[c]["out_p"]).astype(np.float32)
    return out


# revision 17
# speedup vs baseline: 1.0206x; 1.0206x over previous
"""GQA attention kernel for Trainium2, sharded over 8 NeuronCores.

Sharding: core c = b*4 + g handles batch b and GQA group g (4 query heads
+ 1 KV head). Wq/Wk/Wv column-sharded per group, Wo row-sharded; the host
sums the 4 per-group partial outputs per batch (partials in bf16).

Device layout:
  - x is passed transposed (xT [D, S]) so Q^T/K^T/V^T project directly
    into [head_dim, S] layout (head_dim on partitions); V is then
    transposed back to natural [S, head_dim] via the DMA xbar.
  - Q/K head dims are de-interleaved host-side (even dims then odd dims)
    by permuting Wq/Wk columns; scores are invariant to a shared
    permutation of Q/K dims.  RoPE is 4 DVE ops per [128,512] chunk
    using pre-duplicated cos ([c;c]) and pre-signed sin ([-s;+s]).
  - Attention computes scoresT [key, query]; softmax exp output is
    directly the lhs^T operand for the P@V matmul.  Scores for two
    adjacent key chunks share one [128,2,512] PSUM tile so exp runs at
    free-dim 1024.  Probabilities/denominators in bf16.
  - Causal banding at 256-query granularity: diagonal chunk pairs only
    compute the allowed query range; triangular masks finish the job
    (gpsimd + DVE).
  - Denominator: bf16 chunk accumulation (DVE 2x), ones-matmul partition
    reduce, reciprocal_approx_fast, gpsimd partition_broadcast.
  - All matmuls 16-bit.  Projection and output-projection matmuls are
    emitted interleaved into the attention loop ("fillers") so the PE
    FIFO never idles behind the ACT-bound exp chain and the HAM clock
    stays warm; leftover fillers carry across q-blocks.
  - DMA order at startup: x chunks race the Wq chunks so the first
    projection matmuls start ~3 us in; Wo loads last.
"""

import sys

if "/opt/trn_rl_repo" not in sys.path:
    sys.path.insert(0, "/opt/trn_rl_repo")

import numpy as np
import ml_dtypes

import concourse.bass as bass
import concourse.bacc as bacc
import concourse.tile as tile
from concourse import mybir
from concourse.bass_utils import run_bass_kernel_spmd

B = 2
S = 2048
D = 2048
N_HEADS = 16
N_KV = 4
DH = 128
NH = 4  # query heads per core
N_CORES = 8

INV_SQRT_DH = 1.0 / np.sqrt(DH)
F32 = mybir.dt.float32
BF16 = mybir.dt.bfloat16


def build_program(s=S, d=D):
    """Per-core program: 4 query heads + 1 KV head of causal GQA."""
    kc_n = d // 128       # contraction chunks
    qb_n = s // 512       # q-blocks / s-chunks

    nc = bacc.Bacc("TRN2", target_bir_lowering=False, debug=False,
                   num_devices=N_CORES)
    xT = nc.declare_dram_parameter("xT", [d, s], BF16, isOutput=False)
    wq = nc.declare_dram_parameter("wq", [d, NH * DH], BF16, isOutput=False)
    wkv = nc.declare_dram_parameter("wkv", [d, 2 * DH], BF16, isOutput=False)
    wo = nc.declare_dram_parameter("wo", [NH * DH, d], BF16, isOutput=False)
    cosD = nc.declare_dram_parameter("cosD", [128, s], BF16, isOutput=False)
    sinS = nc.declare_dram_parameter("sinS", [128, s], BF16, isOutput=False)
    maskA = nc.declare_dram_parameter("maskA", [128, 128], BF16, isOutput=False)
    maskB = nc.declare_dram_parameter("maskB", [128, 256], BF16, isOutput=False)
    out_p = nc.declare_dram_parameter("out_p", [s, d], BF16, isOutput=True)

    with tile.TileContext(nc) as tc:
        with (
            tc.tile_pool(name="const", bufs=1) as cpool,
            tc.tile_pool(name="xp", bufs=1) as xpool,
            tc.tile_pool(name="act", bufs=1) as apool,
            tc.tile_pool(name="tmp", bufs=1) as tpool,
            tc.tile_pool(name="psum", bufs=1, space="PSUM") as pp,
        ):
            xv = xT.rearrange("(n p) m -> p n m", p=128)
            wqv = wq.rearrange("(n p) m -> p n m", p=128)

            # ---- startup loads: x(sc0) races wq; heavy/late tensors after
            xt0 = xpool.tile([128, kc_n, 512], BF16, tag="xt", bufs=2,
                             name="xt0")
            wq_sb = cpool.tile([128, kc_n, NH * DH], BF16, tag="wq")
            for j4 in range(0, kc_n, 4):
                nc.sync.dma_start(xt0[:, j4:j4 + 4, :], xv[:, j4:j4 + 4, 0:512])
                nc.sync.dma_start(wq_sb[:, j4:j4 + 4, :], wqv[:, j4:j4 + 4, :])
            cos_sb = cpool.tile([128, s], BF16, tag="cos")
            nc.sync.dma_start(cos_sb[:], cosD[:])
            sin_sb = cpool.tile([128, s], BF16, tag="sin")
            nc.sync.dma_start(sin_sb[:], sinS[:])
            wkv_sb = cpool.tile([128, kc_n, 2 * DH], BF16, tag="wkv")
            nc.sync.dma_start(wkv_sb[:], wkv.rearrange("(n p) m -> p n m", p=128))
            mA = cpool.tile([128, 128], BF16, tag="mA")
            nc.sync.dma_start(mA[:], maskA[:])
            mB = cpool.tile([128, 256], BF16, tag="mB")
            nc.sync.dma_start(mB[:], maskB[:])
            ones_col = cpool.tile([128, 1], BF16, tag="ones_col")
            nc.vector.memset(ones_col[:], 1.0)
            wo_sb = cpool.tile([128, NH, d], BF16, tag="wo")
            nc.sync.dma_start(wo_sb[:], wo.rearrange("(n p) m -> p n m", p=128))

            # ---- persistent activations ----
            ktr = apool.tile([128, s], BF16, tag="ktr")
            qtr = {}   # (h, qb) -> tile
            v_sb = {}  # st -> tile
            otr = {}   # (h, qb) -> tile

            def rope(dst, src_psum, sc_i):
                """dst [128,512] bf16 = rope(src) with de-interleaved halves.

                src rows 0:64 = even dims (a), 64:128 = odd dims (b).
                Stage the psum to bf16 SBUF on ACT, then 4 DVE ops at
                bf16 2x rate.  cos_sb = [c; c], sin_sb = [+s; -s]:
                  t1 = qc * cos_sb ; t2 = swap(qc) * sin_sb ; dst = t1+t2
                (swap realized by partition-shifted reads; the sign lives
                in the sin half the INPUT row came from.)
                """
                c = cos_sb[:, sc_i * 512:(sc_i + 1) * 512]
                sg = sin_sb[:, sc_i * 512:(sc_i + 1) * 512]
                qc = tpool.tile([128, 512], BF16, tag="qc", bufs=2)
                nc.scalar.copy(qc[:], src_psum[:])
                t1 = tpool.tile([128, 512], BF16, tag="t1", bufs=2)
                t2 = tpool.tile([128, 512], BF16, tag="t2", bufs=2)
                nc.vector.tensor_mul(t1[:], qc[:], c)
                nc.vector.tensor_mul(t2[0:64, :], qc[64:128, :], sg[64:128, :])
                nc.vector.tensor_mul(t2[64:128, :], qc[0:64, :], sg[0:64, :])
                nc.vector.tensor_add(dst[:], t1[:], t2[:])

            def gen_proj(sc_i):
                """Projection phase for s-chunk sc_i; yields after each PE op."""
                if sc_i == 0:
                    xt = xt0
                else:
                    xt = xpool.tile([128, kc_n, 512], BF16, tag="xt", bufs=2,
                                    name=f"xt{sc_i}")
                    for j4 in range(0, kc_n, 4):
                        nc.sync.dma_start(
                            xt[:, j4:j4 + 4, :],
                            xv[:, j4:j4 + 4, sc_i * 512:(sc_i + 1) * 512])
                # 6 single-psum groups: q0..q3, k, vT (pp rotation overlaps
                # group i+1's matmuls with rope/copy of group i)
                for hh in range(NH + 2):
                    ps = pp.tile([128, 512], F32, tag="pp", bufs=2,
                                 name=f"pj{sc_i}_{hh}")
                    for kc in range(kc_n):
                        if hh < NH:
                            lhsT = wq_sb[:, kc, hh * DH:(hh + 1) * DH]
                        elif hh == NH:
                            lhsT = wkv_sb[:, kc, 0:DH]
                        else:
                            lhsT = wkv_sb[:, kc, DH:2 * DH]
                        nc.tensor.matmul(ps[:], lhsT, xt[:, kc, :],
                                         start=(kc == 0), stop=(kc == kc_n - 1))
                        yield
                    if hh < NH:
                        qtr[(hh, sc_i)] = apool.tile(
                            [128, 512], BF16, tag=f"qtr{hh}", bufs=2,
                            name=f"qtr{hh}_{sc_i}")
                        rope(qtr[(hh, sc_i)][:], ps, sc_i)
                    elif hh == NH:
                        rope(ktr[:, sc_i * 512:(sc_i + 1) * 512], ps, sc_i)
                    else:
                        # vT [dh, 512] -> copy to SBUF -> xbar-transpose to
                        # natural [s, dh] 128-blocks
                        vt_sb = tpool.tile([128, 512], BF16, tag="vt", bufs=2,
                                           name=f"vt{sc_i}")
                        nc.scalar.copy(vt_sb[:], ps[:])
                        for stl in range(4):
                            st = sc_i * 4 + stl
                            v_sb[st] = apool.tile([128, DH], BF16,
                                                  tag=f"v{st}", name=f"v{st}")
                            nc.sync.dma_start_transpose(
                                v_sb[st][:],
                                vt_sb[:, stl * 128:(stl + 1) * 128])

            def gen_outproj(qb, alt_psum=False):
                """Output projection for q-block qb; yields after each PE op."""
                for stl in range(4):
                    st = 4 * qb + stl
                    for dm in range(4):
                        if alt_psum and (stl * 4 + dm) % 2 == 1:
                            wopt = pp.tile([128, 2, 512], F32, tag="sc",
                                           bufs=2, name=f"wop{st}_{dm}")
                            wop = wopt[:, 0, :]
                        else:
                            wop = pp.tile([128, 512], F32, tag="pp", bufs=2,
                                          name=f"wop{st}_{dm}")[:]
                        for h in range(NH):
                            nc.tensor.matmul(
                                wop,
                                otr[(h, qb)][:, stl * 128:(stl + 1) * 128],
                                wo_sb[:, h, dm * 512:(dm + 1) * 512],
                                start=(h == 0), stop=(h == NH - 1))
                            yield
                        osb = tpool.tile([128, 512], BF16, tag="osb", bufs=4,
                                         name=f"osb{st}_{dm}")
                        if (st + dm) % 2 == 0:
                            nc.vector.tensor_copy(osb[:], wop)
                        else:
                            nc.scalar.copy(osb[:], wop)
                        nc.sync.dma_start(
                            out_p[st * 128:(st + 1) * 128,
                                  dm * 512:(dm + 1) * 512], osb[:])

            fillers = []

            def pull(n):
                for _ in range(n):
                    while fillers:
                        try:
                            next(fillers[0])
                            break
                        except StopIteration:
                            fillers.pop(0)
                    else:
                        return

            def drain(gen):
                for _ in gen:
                    pass

            def attn(qb):
                """Attention for q-block qb (4 heads), pulling PE fillers."""
                npair = 2 * (qb + 1)
                pn = 2
                deferred = [None]

                def flush_deferred():
                    if deferred[0] is not None:
                        deferred[0]()
                        deferred[0] = None

                for h in range(NH):
                    q_t = qtr[(h, qb)]
                    l_acc = tpool.tile([128, 512], BF16, tag="lacc", bufs=2,
                                       name=f"lacc{h}_{qb}")
                    otp = pp.tile([128, 512], F32, tag="ot", bufs=2,
                                  name=f"otp{h}_{qb}")
                    pts = []
                    # scores + exp + mask + denominator accumulate
                    for j in range(npair):
                        jd = j - 2 * qb  # >=0 on diagonal pairs
                        qo = 256 * jd if jd > 0 else 0
                        scp = pp.tile([128, 2, 512], F32, tag="sc", bufs=2,
                                      name=f"scp{h}_{qb}_{j}")
                        nc.tensor.matmul(scp[:, 0, qo:],
                                         ktr[:, (2 * j) * 128:(2 * j + 1) * 128],
                                         q_t[:, qo:], start=True, stop=True)
                        pull(pn)
                        nc.tensor.matmul(scp[:, 1, qo:],
                                         ktr[:, (2 * j + 1) * 128:(2 * j + 2) * 128],
                                         q_t[:, qo:], start=True, stop=True)
                        pull(pn)
                        pt = tpool.tile([128, 2, 512], BF16, tag="pt", bufs=9,
                                        name=f"pt{h}_{qb}_{j}")
                        pts.append((pt, qo))
                        nc.scalar.activation(
                            pt[:, :, qo:], scp[:, :, qo:],
                            mybir.ActivationFunctionType.Exp,
                            scale=float(INV_SQRT_DH))
                        if jd >= 0:
                            # chunk 2j: triangle in cols [qo, qo+128)
                            nc.vector.tensor_mul(pt[:, 0, qo:qo + 128],
                                                 pt[:, 0, qo:qo + 128], mA[:])
                            # chunk 2j+1: first 128 cols dead + triangle
                            nc.vector.tensor_mul(pt[:, 1, qo:qo + 256],
                                                 pt[:, 1, qo:qo + 256], mB[:])
                        if j == 0:
                            nc.vector.tensor_copy(l_acc[:], pt[:, 0, :])
                        else:
                            nc.vector.tensor_add(l_acc[:, qo:], l_acc[:, qo:],
                                                 pt[:, 0, qo:])
                        nc.vector.tensor_add(l_acc[:, qo:], l_acc[:, qo:],
                                             pt[:, 1, qo:])
                    # previous head's normalize now (its all-reduce is done,
                    # so these DVE ops don't stall the vector FIFO)
                    flush_deferred()
                    # P @ V (dense on PE)
                    for j in range(npair):
                        pt, qo = pts[j]
                        nc.tensor.matmul(otp[:, qo:], v_sb[2 * j][:],
                                         pt[:, 0, qo:], start=(j == 0),
                                         stop=False)
                        nc.tensor.matmul(otp[:, qo:], v_sb[2 * j + 1][:],
                                         pt[:, 1, qo:], start=False,
                                         stop=(j == npair - 1))
                    otr[(h, qb)] = apool.tile([128, 512], BF16, tag=f"otr{h}",
                                              bufs=2, name=f"otr{h}_{qb}")
                    if qb == qb_n - 1 and h == NH - 1:
                        # last head of the kernel: shortest-latency tail so
                        # the final output projection isn't held up
                        lpt = pp.tile([128, 2, 512], F32, tag="sc", bufs=2,
                                      name=f"lp{h}_{qb}")
                        nc.tensor.matmul(lpt[0:1, 0, :], ones_col[:],
                                         l_acc[:], start=True, stop=True)
                        rl = tpool.tile([1, 512], F32, tag="rl", bufs=2,
                                        name=f"rl{h}_{qb}")
                        nc.vector.reciprocal_approx_fast(rl[:], lpt[0:1, 0, :])
                        rlb = tpool.tile([128, 512], F32, tag="rlbs", bufs=2,
                                         name=f"rlbs{h}_{qb}")
                        nc.gpsimd.partition_broadcast(rlb[:], rl[:])
                        nc.vector.tensor_mul(otr[(h, qb)][:], otp[:], rlb[:])
                    else:
                        # denominator all-reduce+broadcast on gpsimd; defer
                        # the reciprocal + normalize to the next window
                        lred = tpool.tile([128, 512], F32, tag="lred", bufs=2,
                                          name=f"lred{h}_{qb}")
                        nc.gpsimd.partition_all_reduce(
                            lred[:], l_acc[:], 128, bass.bass_isa.ReduceOp.add)

                        def tail(h=h, lred=lred, otp=otp):
                            rlb = tpool.tile([128, 512], F32, tag="rlbs",
                                             bufs=2, name=f"rlbs{h}_{qb}")
                            nc.vector.reciprocal_approx_fast(rlb[:], lred[:])
                            nc.vector.tensor_mul(otr[(h, qb)][:], otp[:],
                                                 rlb[:])

                        deferred[0] = tail
                    pull(2)
                flush_deferred()

            # ---- main schedule ----
            drain(gen_proj(0))
            for qb in range(qb_n):
                if qb + 1 < qb_n:
                    g = gen_proj(qb + 1)
                    next(g, None)  # prime: emit DMAs early
                    # proj first in the pull order: outproj fillers bank up
                    # across q-blocks so attn(3) has enough PE filler work
                    fillers.insert(0, g)
                    attn(qb)
                    # proj(qb+1) must fully drain before attn(qb+1): its
                    # rope outputs feed the next q-block's scores.  (If g
                    # is already exhausted this is a no-op; pull() will
                    # pop the spent generator from the list later.)
                    drain(g)
                    fillers.append(gen_outproj(qb))
                else:
                    attn(qb)
            while fillers:
                drain(fillers.pop(0))
            drain(gen_outproj(qb_n - 1, alt_psum=True))

    nc.compile()
    return nc


_PROGRAM = None


def _get_program():
    global _PROGRAM
    if _PROGRAM is None:
        _PROGRAM = build_program()
    return _PROGRAM


_DEINT = np.concatenate([np.arange(0, DH, 2), np.arange(1, DH, 2)])


def make_in_maps(x, rope_cos, rope_sin, Wq, Wk, Wv, Wo, s=S):
    cosT = rope_cos[:s].T.astype(np.float32)   # [64, s]
    sinT = rope_sin[:s].T.astype(np.float32)
    cosD = np.concatenate([cosT, cosT], axis=0).astype(ml_dtypes.bfloat16)
    sinS = np.concatenate([sinT, -sinT], axis=0).astype(ml_dtypes.bfloat16)
    p = np.arange(128)[:, None]
    maskA = (np.arange(128)[None, :] >= p).astype(ml_dtypes.bfloat16)
    maskB = (np.arange(256)[None, :] >= p + 128).astype(ml_dtypes.bfloat16)
    in_maps = []
    for c in range(N_CORES):
        b, g = divmod(c, 4)
        xTc = np.ascontiguousarray(x[b].T.astype(ml_dtypes.bfloat16))
        wq_cols = [
            Wq[:, (g * NH + j) * DH:(g * NH + j + 1) * DH][:, _DEINT]
            for j in range(NH)
        ]
        wq_c = np.ascontiguousarray(
            np.concatenate(wq_cols, axis=1).astype(ml_dtypes.bfloat16))
        wk_c = Wk[:, g * DH:(g + 1) * DH][:, _DEINT]
        wv_c = Wv[:, g * DH:(g + 1) * DH]
        wkv_c = np.ascontiguousarray(
            np.concatenate([wk_c, wv_c], axis=1).astype(ml_dtypes.bfloat16))
        wo_c = np.ascontiguousarray(
            Wo[g * NH * DH:(g + 1) * NH * DH, :].astype(ml_dtypes.bfloat16))
        in_maps.append({
            "xT": xTc, "wq": wq_c, "wkv": wkv_c, "wo": wo_c,
            "cosD": np.ascontiguousarray(cosD),
            "sinS": np.ascontiguousarray(sinS),
            "maskA": maskA, "maskB": maskB,
        })
    return in_maps


def kernel(x, rope_cos, rope_sin, Wq, Wk, Wv, Wo):
    nc = _get_program()
    in_maps = make_in_maps(x, rope_cos, rope_sin, Wq, Wk, Wv, Wo)
    res = run_bass_kernel_spmd(nc, in_maps, list(range(N_CORES)))
    out = np.zeros((B, S, D), dtype=np.float32)
    for c in range(N_CORES):
        b, g = divmod(c, 4)
        out[b] += np.asarray(res.results[c]["out_p"]).astype(np.float32)
    return out


# revision 20
# speedup vs baseline: 1.0240x; 1.0033x over previous
"""GQA attention kernel for Trainium2, sharded over 8 NeuronCores.

Sharding: core c = b*4 + g handles batch b and GQA group g (4 query heads
+ 1 KV head). Wq/Wk/Wv column-sharded per group, Wo row-sharded; the host
sums the 4 per-group partial outputs per batch (partials in bf16).

Device layout:
  - x is passed transposed (xT [D, S]) so Q^T/K^T/V^T project directly
    into [head_dim, S] layout (head_dim on partitions); V is then
    transposed back to natural [S, head_dim] via the DMA xbar.
  - Q/K head dims are de-interleaved host-side (even dims then odd dims)
    by permuting Wq/Wk columns; scores are invariant to a shared
    permutation of Q/K dims.  RoPE is 4 DVE ops per [128,512] chunk
    using pre-duplicated cos ([c;c]) and pre-signed sin ([-s;+s]).
  - Attention computes scoresT [key, query]; softmax exp output is
    directly the lhs^T operand for the P@V matmul.  Scores for two
    adjacent key chunks share one [128,2,512] PSUM tile so exp runs at
    free-dim 1024.  Probabilities/denominators in bf16.
  - Causal banding at 256-query granularity: diagonal chunk pairs only
    compute the allowed query range; triangular masks finish the job
    (gpsimd + DVE).
  - Denominator: bf16 chunk accumulation (DVE 2x), ones-matmul partition
    reduce, reciprocal_approx_fast, gpsimd partition_broadcast.
  - All matmuls 16-bit.  Projection and output-projection matmuls are
    emitted interleaved into the attention loop ("fillers") so the PE
    FIFO never idles behind the ACT-bound exp chain and the HAM clock
    stays warm; leftover fillers carry across q-blocks.
  - DMA order at startup: x chunks race the Wq chunks so the first
    projection matmuls start ~3 us in; Wo loads last.
"""

import sys

if "/opt/trn_rl_repo" not in sys.path:
    sys.path.insert(0, "/opt/trn_rl_repo")

import numpy as np
import ml_dtypes

import concourse.bass as bass
import concourse.bacc as bacc
import concourse.tile as tile
from concourse import mybir
from concourse.bass_utils import run_bass_kernel_spmd

B = 2
S = 2048
D = 2048
N_HEADS = 16
N_KV = 4
DH = 128
NH = 4  # query heads per core
N_CORES = 8

INV_SQRT_DH = 1.0 / np.sqrt(DH)
F32 = mybir.dt.float32
BF16 = mybir.dt.bfloat16


def build_program(s=S, d=D):
    """Per-core program: 4 query heads + 1 KV head of causal GQA."""
    kc_n = d // 128       # contraction chunks
    qb_n = s // 512       # q-blocks / s-chunks

    nc = bacc.Bacc("TRN2", target_bir_lowering=False, debug=False,
                   num_devices=N_CORES)
    xT = nc.declare_dram_parameter("xT", [d, s], BF16, isOutput=False)
    wq = nc.declare_dram_parameter("wq", [d, NH * DH], BF16, isOutput=False)
    wkv = nc.declare_dram_parameter("wkv", [d, 2 * DH], BF16, isOutput=False)
    wo = nc.declare_dram_parameter("wo", [NH * DH, d], BF16, isOutput=False)
    cosD = nc.declare_dram_parameter("cosD", [128, s], BF16, isOutput=False)
    sinS = nc.declare_dram_parameter("sinS", [128, s], BF16, isOutput=False)
    maskA = nc.declare_dram_parameter("maskA", [128, 128], BF16, isOutput=False)
    maskB = nc.declare_dram_parameter("maskB", [128, 256], BF16, isOutput=False)
    out_p = nc.declare_dram_parameter("out_p", [s, d], BF16, isOutput=True)

    with tile.TileContext(nc) as tc:
        with (
            tc.tile_pool(name="const", bufs=1) as cpool,
            tc.tile_pool(name="xp", bufs=1) as xpool,
            tc.tile_pool(name="act", bufs=1) as apool,
            tc.tile_pool(name="tmp", bufs=1) as tpool,
            tc.tile_pool(name="psum", bufs=1, space="PSUM") as pp,
        ):
            xv = xT.rearrange("(n p) m -> p n m", p=128)
            wqv = wq.rearrange("(n p) m -> p n m", p=128)

            # ---- startup loads: x(sc0) races wq; heavy/late tensors after
            xt0 = xpool.tile([128, kc_n, 512], BF16, tag="xt", bufs=2,
                             name="xt0")
            wq_sb = cpool.tile([128, kc_n, NH * DH], BF16, tag="wq")
            for j4 in range(0, kc_n, 4):
                nc.sync.dma_start(xt0[:, j4:j4 + 4, :], xv[:, j4:j4 + 4, 0:512])
                nc.sync.dma_start(wq_sb[:, j4:j4 + 4, :], wqv[:, j4:j4 + 4, :])
            cos_sb = cpool.tile([128, s], BF16, tag="cos")
            nc.sync.dma_start(cos_sb[:], cosD[:])
            sin_sb = cpool.tile([128, s], BF16, tag="sin")
            nc.sync.dma_start(sin_sb[:], sinS[:])
            wkv_sb = cpool.tile([128, kc_n, 2 * DH], BF16, tag="wkv")
            nc.sync.dma_start(wkv_sb[:], wkv.rearrange("(n p) m -> p n m", p=128))
            mA = cpool.tile([128, 128], BF16, tag="mA")
            nc.sync.dma_start(mA[:], maskA[:])
            mB = cpool.tile([128, 256], BF16, tag="mB")
            nc.sync.dma_start(mB[:], maskB[:])
            ones_col = cpool.tile([128, 1], BF16, tag="ones_col")
            nc.vector.memset(ones_col[:], 1.0)
            wo_sb = cpool.tile([128, NH, d], BF16, tag="wo")
            nc.sync.dma_start(wo_sb[:], wo.rearrange("(n p) m -> p n m", p=128))

            # ---- persistent activations ----
            ktr = apool.tile([128, s], BF16, tag="ktr")
            qtr = {}   # (h, qb) -> tile
            v_sb = {}  # st -> tile
            otr = {}   # (h, qb) -> tile

            def rope(dst, src_psum, sc_i):
                """dst [128,512] bf16 = rope(src) with de-interleaved halves.

                src rows 0:64 = even dims (a), 64:128 = odd dims (b).
                Stage the psum to bf16 SBUF on ACT, then 4 DVE ops at
                bf16 2x rate.  cos_sb = [c; c], sin_sb = [+s; -s]:
                  t1 = qc * cos_sb ; t2 = swap(qc) * sin_sb ; dst = t1+t2
                (swap realized by partition-shifted reads; the sign lives
                in the sin half the INPUT row came from.)
                """
                c = cos_sb[:, sc_i * 512:(sc_i + 1) * 512]
                sg = sin_sb[:, sc_i * 512:(sc_i + 1) * 512]
                qc = tpool.tile([128, 512], BF16, tag="qc", bufs=2)
                nc.scalar.copy(qc[:], src_psum[:])
                t1 = tpool.tile([128, 512], BF16, tag="t1", bufs=2)
                t2 = tpool.tile([128, 512], BF16, tag="t2", bufs=2)
                nc.vector.tensor_mul(t1[:], qc[:], c)
                nc.vector.tensor_mul(t2[0:64, :], qc[64:128, :], sg[64:128, :])
                nc.vector.tensor_mul(t2[64:128, :], qc[0:64, :], sg[0:64, :])
                nc.vector.tensor_add(dst[:], t1[:], t2[:])

            def gen_proj(sc_i):
                """Projection phase for s-chunk sc_i; yields after each PE op."""
                if sc_i == 0:
                    xt = xt0
                else:
                    xt = xpool.tile([128, kc_n, 512], BF16, tag="xt", bufs=2,
                                    name=f"xt{sc_i}")
                    for j4 in range(0, kc_n, 4):
                        nc.sync.dma_start(
                            xt[:, j4:j4 + 4, :],
                            xv[:, j4:j4 + 4, sc_i * 512:(sc_i + 1) * 512])
                # 6 single-psum groups: q0..q3, k, vT (pp rotation overlaps
                # group i+1's matmuls with rope/copy of group i).  At sc0
                # the x/w stream is still arriving: emit heads 0+1
                # interleaved per kc-block so the PE has work per chunk.
                def emit_group(hh, ps, kcs):
                    for kc in kcs:
                        if hh < NH:
                            lhsT = wq_sb[:, kc, hh * DH:(hh + 1) * DH]
                        elif hh == NH:
                            lhsT = wkv_sb[:, kc, 0:DH]
                        else:
                            lhsT = wkv_sb[:, kc, DH:2 * DH]
                        nc.tensor.matmul(ps[:], lhsT, xt[:, kc, :],
                                         start=(kc == 0), stop=(kc == kc_n - 1))
                        yield

                ps_h = {}
                for hh in range(NH + 2):
                    if sc_i == 0 and hh in (0, 1):
                        if hh == 0:
                            ps_h[0] = pp.tile([128, 512], F32, tag="pp",
                                              bufs=2, name=f"pj{sc_i}_0")
                            ps_h[1] = pp.tile([128, 512], F32, tag="pp",
                                              bufs=2, name=f"pj{sc_i}_1")
                            for j4 in range(0, kc_n, 4):
                                for g in (emit_group(0, ps_h[0], range(j4, j4 + 4)),
                                          emit_group(1, ps_h[1], range(j4, j4 + 4))):
                                    yield from g
                        ps = ps_h[hh]
                    else:
                        ps = pp.tile([128, 512], F32, tag="pp", bufs=2,
                                     name=f"pj{sc_i}_{hh}")
                        yield from emit_group(hh, ps, range(kc_n))
                    if hh < NH:
                        qtr[(hh, sc_i)] = apool.tile(
                            [128, 512], BF16, tag=f"qtr{hh}", bufs=2,
                            name=f"qtr{hh}_{sc_i}")
                        rope(qtr[(hh, sc_i)][:], ps, sc_i)
                    elif hh == NH:
                        rope(ktr[:, sc_i * 512:(sc_i + 1) * 512], ps, sc_i)
                    else:
                        # vT [dh, 512] -> copy to SBUF -> xbar-transpose to
                        # natural [s, dh] 128-blocks
                        vt_sb = tpool.tile([128, 512], BF16, tag="vt", bufs=2,
                                           name=f"vt{sc_i}")
                        nc.scalar.copy(vt_sb[:], ps[:])
                        for stl in range(4):
                            st = sc_i * 4 + stl
                            v_sb[st] = apool.tile([128, DH], BF16,
                                                  tag=f"v{st}", name=f"v{st}")
                            nc.sync.dma_start_transpose(
                                v_sb[st][:],
                                vt_sb[:, stl * 128:(stl + 1) * 128])

            def gen_outproj(qb, alt_psum=False):
                """Output projection for q-block qb; yields after each PE op."""
                for stl in range(4):
                    st = 4 * qb + stl
                    for dm in range(4):
                        if alt_psum and (stl * 4 + dm) % 2 == 1:
                            wopt = pp.tile([128, 2, 512], F32, tag="sc",
                                           bufs=2, name=f"wop{st}_{dm}")
                            wop = wopt[:, 0, :]
                        else:
                            wop = pp.tile([128, 512], F32, tag="pp", bufs=2,
                                          name=f"wop{st}_{dm}")[:]
                        for h in range(NH):
                            nc.tensor.matmul(
                                wop,
                                otr[(h, qb)][:, stl * 128:(stl + 1) * 128],
                                wo_sb[:, h, dm * 512:(dm + 1) * 512],
                                start=(h == 0), stop=(h == NH - 1))
                            yield
                        osb = tpool.tile([128, 512], BF16, tag="osb", bufs=4,
                                         name=f"osb{st}_{dm}")
                        if (st + dm) % 2 == 0:
                            nc.vector.tensor_copy(osb[:], wop)
                        else:
                            nc.scalar.copy(osb[:], wop)
                        nc.sync.dma_start(
                            out_p[st * 128:(st + 1) * 128,
                                  dm * 512:(dm + 1) * 512], osb[:])

            fillers = []

            def pull(n):
                for _ in range(n):
                    while fillers:
                        try:
                            next(fillers[0])
                            break
                        except StopIteration:
                            fillers.pop(0)
                    else:
                        return

            def drain(gen):
                for _ in gen:
                    pass

            def attn(qb):
                """Attention for q-block qb (4 heads), pulling PE fillers."""
                npair = 2 * (qb + 1)
                pn = 2
                deferred = [None]

                def flush_deferred():
                    if deferred[0] is not None:
                        deferred[0]()
                        deferred[0] = None

                for h in range(NH):
                    q_t = qtr[(h, qb)]
                    l_acc = tpool.tile([128, 512], BF16, tag="lacc", bufs=2,
                                       name=f"lacc{h}_{qb}")
                    otp = pp.tile([128, 512], F32, tag="ot", bufs=2,
                                  name=f"otp{h}_{qb}")
                    pts = []
                    # scores + exp + mask + denominator accumulate
                    for j in range(npair):
                        jd = j - 2 * qb  # >=0 on diagonal pairs
                        qo = 256 * jd if jd > 0 else 0
                        scp = pp.tile([128, 2, 512], F32, tag="sc", bufs=2,
                                      name=f"scp{h}_{qb}_{j}")
                        nc.tensor.matmul(scp[:, 0, qo:],
                                         ktr[:, (2 * j) * 128:(2 * j + 1) * 128],
                                         q_t[:, qo:], start=True, stop=True)
                        pull(pn)
                        nc.tensor.matmul(scp[:, 1, qo:],
                                         ktr[:, (2 * j + 1) * 128:(2 * j + 2) * 128],
                                         q_t[:, qo:], start=True, stop=True)
                        pull(pn)
                        pt = tpool.tile([128, 2, 512], BF16, tag="pt", bufs=12,
                                        name=f"pt{h}_{qb}_{j}")
                        pts.append((pt, qo))
                        nc.scalar.activation(
                            pt[:, :, qo:], scp[:, :, qo:],
                            mybir.ActivationFunctionType.Exp,
                            scale=float(INV_SQRT_DH))
                        if jd >= 0:
                            # chunk 2j: triangle in cols [qo, qo+128)
                            nc.vector.tensor_mul(pt[:, 0, qo:qo + 128],
                                                 pt[:, 0, qo:qo + 128], mA[:])
                            # chunk 2j+1: first 128 cols dead + triangle
                            nc.vector.tensor_mul(pt[:, 1, qo:qo + 256],
                                                 pt[:, 1, qo:qo + 256], mB[:])
                        if j == 0:
                            nc.vector.tensor_copy(l_acc[:], pt[:, 0, :])
                        else:
                            nc.vector.tensor_add(l_acc[:, qo:], l_acc[:, qo:],
                                                 pt[:, 0, qo:])
                        nc.vector.tensor_add(l_acc[:, qo:], l_acc[:, qo:],
                                             pt[:, 1, qo:])
                    # start the denominator reduce now -- l_acc is complete
                    # after the scores loop, so it overlaps the P@V matmuls
                    otr[(h, qb)] = apool.tile([128, 512], BF16, tag=f"otr{h}",
                                              bufs=2, name=f"otr{h}_{qb}")
                    last = qb == qb_n - 1 and h == NH - 1
                    if last:
                        # last head of the kernel: shortest-latency tail so
                        # the final output projection isn't held up
                        lpt = pp.tile([128, 2, 512], F32, tag="sc", bufs=2,
                                      name=f"lp{h}_{qb}")
                        nc.tensor.matmul(lpt[0:1, 0, :], ones_col[:],
                                         l_acc[:], start=True, stop=True)
                        rlb = tpool.tile([128, 512], F32, tag="rlbs", bufs=2,
                                         name=f"rlbs{h}_{qb}")
                        rl = tpool.tile([1, 512], F32, tag="rl", bufs=2,
                                        name=f"rl{h}_{qb}")
                        nc.vector.reciprocal_approx_fast(rl[:], lpt[0:1, 0, :])
                        nc.gpsimd.partition_broadcast(rlb[:], rl[:])
                    else:
                        lred = tpool.tile([128, 512], F32, tag="lred", bufs=2,
                                          name=f"lred{h}_{qb}")
                        nc.gpsimd.partition_all_reduce(
                            lred[:], l_acc[:], 128, bass.bass_isa.ReduceOp.add)
                    # previous head's normalize now (its reduce is done,
                    # so these DVE ops don't stall the vector FIFO)
                    flush_deferred()
                    # P @ V (dense on PE)
                    for j in range(npair):
                        pt, qo = pts[j]
                        nc.tensor.matmul(otp[:, qo:], v_sb[2 * j][:],
                                         pt[:, 0, qo:], start=(j == 0),
                                         stop=False)
                        nc.tensor.matmul(otp[:, qo:], v_sb[2 * j + 1][:],
                                         pt[:, 1, qo:], start=False,
                                         stop=(j == npair - 1))
                    if last:
                        nc.vector.tensor_mul(otr[(h, qb)][:], otp[:], rlb[:])
                    else:
                        def tail(h=h, lred=lred, otp=otp):
                            rlb2 = tpool.tile([128, 512], F32, tag="rlbs",
                                              bufs=2, name=f"rlbs{h}_{qb}")
                            nc.vector.reciprocal_approx_fast(rlb2[:], lred[:])
                            nc.vector.tensor_mul(otr[(h, qb)][:], otp[:],
                                                 rlb2[:])

                        deferred[0] = tail
                    pull(2)
                flush_deferred()

            # ---- main schedule ----
            drain(gen_proj(0))
            for qb in range(qb_n):
                if qb + 1 < qb_n:
                    g = gen_proj(qb + 1)
                    next(g, None)  # prime: emit DMAs early
                    # proj first in the pull order: outproj fillers bank up
                    # across q-blocks so attn(3) has enough PE filler work
                    fillers.insert(0, g)
                    attn(qb)
                    # proj(qb+1) must fully drain before attn(qb+1): its
                    # rope outputs feed the next q-block's scores.  (If g
                    # is already exhausted this is a no-op; pull() will
                    # pop the spent generator from the list later.)
                    drain(g)
                    fillers.append(gen_outproj(qb))
                else:
                    attn(qb)
            while fillers:
                drain(fillers.pop(0))
            drain(gen_outproj(qb_n - 1, alt_psum=True))

    nc.compile()
    return nc


_PROGRAM = None


def _get_program():
    global _PROGRAM
    if _PROGRAM is None:
        _PROGRAM = build_program()
    return _PROGRAM


_DEINT = np.concatenate([np.arange(0, DH, 2), np.arange(1, DH, 2)])


def make_in_maps(x, rope_cos, rope_sin, Wq, Wk, Wv, Wo, s=S):
    cosT = rope_cos[:s].T.astype(np.float32)   # [64, s]
    sinT = rope_sin[:s].T.astype(np.float32)
    cosD = np.concatenate([cosT, cosT], axis=0).astype(ml_dtypes.bfloat16)
    sinS = np.concatenate([sinT, -sinT], axis=0).astype(ml_dtypes.bfloat16)
    p = np.arange(128)[:, None]
    maskA = (np.arange(128)[None, :] >= p).astype(ml_dtypes.bfloat16)
    maskB = (np.arange(256)[None, :] >= p + 128).astype(ml_dtypes.bfloat16)
    in_maps = []
    for c in range(N_CORES):
        b, g = divmod(c, 4)
        xTc = np.ascontiguousarray(x[b].T.astype(ml_dtypes.bfloat16))
        wq_cols = [
            Wq[:, (g * NH + j) * DH:(g * NH + j + 1) * DH][:, _DEINT]
            for j in range(NH)
        ]
        wq_c = np.ascontiguousarray(
            np.concatenate(wq_cols, axis=1).astype(ml_dtypes.bfloat16))
        wk_c = Wk[:, g * DH:(g + 1) * DH][:, _DEINT]
        wv_c = Wv[:, g * DH:(g + 1) * DH]
        wkv_c = np.ascontiguousarray(
            np.concatenate([wk_c, wv_c], axis=1).astype(ml_dtypes.bfloat16))
        wo_c = np.ascontiguousarray(
            Wo[g * NH * DH:(g + 1) * NH * DH, :].astype(ml_dtypes.bfloat16))
        in_maps.append({
            "xT": xTc, "wq": wq_c, "wkv": wkv_c, "wo": wo_c,
            "cosD": np.ascontiguousarray(cosD),
            "sinS": np.ascontiguousarray(sinS),
            "maskA": maskA, "maskB": maskB,
        })
    return in_maps


def kernel(x, rope_cos, rope_sin, Wq, Wk, Wv, Wo):
    nc = _get_program()
    in_maps = make_in_maps(x, rope_cos, rope_sin, Wq, Wk, Wv, Wo)
    res = run_bass_kernel_spmd(nc, in_maps, list(range(N_CORES)))
    out = np.zeros((B, S, D), dtype=np.float32)
    for c in range(N_CORES):
        b, g = divmod(c, 4)
        out[b] += np.asarray(res.results[c]["out_p"]).astype(np.float32)
    return out


# revision 23
# speedup vs baseline: 1.0406x; 1.0163x over previous
"""GQA attention kernel for Trainium2, sharded over 8 NeuronCores.

Sharding: core c = b*4 + g handles batch b and GQA group g (4 query heads
+ 1 KV head). Wq/Wk/Wv column-sharded per group, Wo row-sharded; the host
sums the 4 per-group partial outputs per batch (partials in bf16).

Device layout:
  - x is passed transposed (xT [D, S]) so Q^T/K^T/V^T project directly
    into [head_dim, S] layout (head_dim on partitions); V is then
    transposed back to natural [S, head_dim] via the DMA xbar.
  - Q/K head dims are de-interleaved host-side (even dims then odd dims)
    by permuting Wq/Wk columns; scores are invariant to a shared
    permutation of Q/K dims.  RoPE is 4 DVE ops per [128,512] chunk
    using pre-duplicated cos ([c;c]) and pre-signed sin ([-s;+s]).
  - Attention computes scoresT [key, query]; softmax exp output is
    directly the lhs^T operand for the P@V matmul.  Scores for two
    adjacent key chunks share one [128,2,512] PSUM tile so exp runs at
    free-dim 1024.  Probabilities/denominators in bf16.
  - Causal banding at 256-query granularity: diagonal chunk pairs only
    compute the allowed query range; triangular masks finish the job
    (gpsimd + DVE).
  - Denominator: bf16 chunk accumulation (DVE 2x), ones-matmul partition
    reduce, reciprocal_approx_fast, gpsimd partition_broadcast.
  - All matmuls 16-bit.  Projection and output-projection matmuls are
    emitted interleaved into the attention loop ("fillers") so the PE
    FIFO never idles behind the ACT-bound exp chain and the HAM clock
    stays warm; leftover fillers carry across q-blocks.
  - DMA order at startup: x chunks race the Wq chunks so the first
    projection matmuls start ~3 us in; Wo loads last.
"""

import sys

if "/opt/trn_rl_repo" not in sys.path:
    sys.path.insert(0, "/opt/trn_rl_repo")

import numpy as np
import ml_dtypes

import concourse.bass as bass
import concourse.bacc as bacc
import concourse.tile as tile
from concourse import mybir
from concourse.bass_utils import run_bass_kernel_spmd

B = 2
S = 2048
D = 2048
N_HEADS = 16
N_KV = 4
DH = 128
NH = 4  # query heads per core
N_CORES = 8

INV_SQRT_DH = 1.0 / np.sqrt(DH)
F32 = mybir.dt.float32
BF16 = mybir.dt.bfloat16


def build_program(s=S, d=D):
    """Per-core program: 4 query heads + 1 KV head of causal GQA."""
    kc_n = d // 128       # contraction chunks
    qb_n = s // 512       # q-blocks / s-chunks

    nc = bacc.Bacc("TRN2", target_bir_lowering=False, debug=False,
                   num_devices=N_CORES)
    xT = nc.declare_dram_parameter("xT", [d, s], BF16, isOutput=False)
    wq = nc.declare_dram_parameter("wq", [d, NH * DH], BF16, isOutput=False)
    wkv = nc.declare_dram_parameter("wkv", [d, 2 * DH], BF16, isOutput=False)
    wo = nc.declare_dram_parameter("wo", [NH * DH, d], BF16, isOutput=False)
    cosD = nc.declare_dram_parameter("cosD", [128, s], BF16, isOutput=False)
    sinS = nc.declare_dram_parameter("sinS", [128, s], BF16, isOutput=False)
    maskA = nc.declare_dram_parameter("maskA", [128, 128], BF16, isOutput=False)
    maskB = nc.declare_dram_parameter("maskB", [128, 256], BF16, isOutput=False)
    out_p = nc.declare_dram_parameter("out_p", [s, d], BF16, isOutput=True)

    with tile.TileContext(nc) as tc:
        with (
            tc.tile_pool(name="const", bufs=1) as cpool,
            tc.tile_pool(name="xp", bufs=1) as xpool,
            tc.tile_pool(name="act", bufs=1) as apool,
            tc.tile_pool(name="tmp", bufs=1) as tpool,
            tc.tile_pool(name="psum", bufs=1, space="PSUM") as pp,
        ):
            xv = xT.rearrange("(n p) m -> p n m", p=128)
            wqv = wq.rearrange("(n p) m -> p n m", p=128)

            # ---- startup loads: x(sc0) races wq; heavy/late tensors after
            xt0 = xpool.tile([128, kc_n, 512], BF16, tag="xt", bufs=2,
                             name="xt0")
            wq_sb = cpool.tile([128, kc_n, NH * DH], BF16, tag="wq")
            for j4 in range(0, kc_n, 4):
                nc.sync.dma_start(xt0[:, j4:j4 + 4, :], xv[:, j4:j4 + 4, 0:512])
                nc.sync.dma_start(wq_sb[:, j4:j4 + 4, :], wqv[:, j4:j4 + 4, :])
            cos_sb = cpool.tile([128, s], BF16, tag="cos")
            nc.sync.dma_start(cos_sb[:], cosD[:])
            sin_sb = cpool.tile([128, s], BF16, tag="sin")
            nc.sync.dma_start(sin_sb[:], sinS[:])
            wkv_sb = cpool.tile([128, kc_n, 2 * DH], BF16, tag="wkv")
            nc.sync.dma_start(wkv_sb[:], wkv.rearrange("(n p) m -> p n m", p=128))
            mA = cpool.tile([128, 128], BF16, tag="mA")
            nc.sync.dma_start(mA[:], maskA[:])
            mB = cpool.tile([128, 256], BF16, tag="mB")
            nc.sync.dma_start(mB[:], maskB[:])
            ones_col = cpool.tile([128, 1], BF16, tag="ones_col")
            nc.vector.memset(ones_col[:], 1.0)
            wo_sb = cpool.tile([128, NH, d], BF16, tag="wo")
            nc.sync.dma_start(wo_sb[:], wo.rearrange("(n p) m -> p n m", p=128))

            # ---- persistent activations ----
            ktr = apool.tile([128, s], BF16, tag="ktr")
            qtr = {}   # (h, qb) -> tile
            v_sb = {}  # st -> tile
            otr = {}   # (h, qb) -> tile

            def rope(dst, src_psum, sc_i):
                """dst [128,512] bf16 = rope(src) with de-interleaved halves.

                src rows 0:64 = even dims (a), 64:128 = odd dims (b).
                Stage the psum to bf16 SBUF on ACT, then 4 DVE ops at
                bf16 2x rate.  cos_sb = [c; c], sin_sb = [+s; -s]:
                  t1 = qc * cos_sb ; t2 = swap(qc) * sin_sb ; dst = t1+t2
                (swap realized by partition-shifted reads; the sign lives
                in the sin half the INPUT row came from.)
                """
                c = cos_sb[:, sc_i * 512:(sc_i + 1) * 512]
                sg = sin_sb[:, sc_i * 512:(sc_i + 1) * 512]
                qc = tpool.tile([128, 512], BF16, tag="qc", bufs=2)
                nc.scalar.copy(qc[:], src_psum[:])
                t1 = tpool.tile([128, 512], BF16, tag="t1", bufs=2)
                t2 = tpool.tile([128, 512], BF16, tag="t2", bufs=2)
                nc.vector.tensor_mul(t1[:], qc[:], c)
                nc.vector.tensor_mul(t2[0:64, :], qc[64:128, :], sg[64:128, :])
                nc.vector.tensor_mul(t2[64:128, :], qc[0:64, :], sg[0:64, :])
                nc.vector.tensor_add(dst[:], t1[:], t2[:])

            def gen_proj(sc_i):
                """Projection phase for s-chunk sc_i; yields after each PE op."""
                if sc_i == 0:
                    xt = xt0
                else:
                    xt = xpool.tile([128, kc_n, 512], BF16, tag="xt", bufs=2,
                                    name=f"xt{sc_i}")
                    for j4 in range(0, kc_n, 4):
                        nc.sync.dma_start(
                            xt[:, j4:j4 + 4, :],
                            xv[:, j4:j4 + 4, sc_i * 512:(sc_i + 1) * 512])
                # 6 single-psum groups: q0..q3, k, vT (pp rotation overlaps
                # group i+1's matmuls with rope/copy of group i).  At sc0
                # the x/w stream is still arriving: emit heads 0+1
                # interleaved per kc-block so the PE has work per chunk.
                def emit_group(hh, ps, kcs):
                    for kc in kcs:
                        if hh < NH:
                            lhsT = wq_sb[:, kc, hh * DH:(hh + 1) * DH]
                        elif hh == NH:
                            lhsT = wkv_sb[:, kc, 0:DH]
                        else:
                            lhsT = wkv_sb[:, kc, DH:2 * DH]
                        nc.tensor.matmul(ps[:], lhsT, xt[:, kc, :],
                                         start=(kc == 0), stop=(kc == kc_n - 1))
                        yield

                ps_h = {}
                for hh in range(NH + 2):
                    if sc_i == 0 and hh in (0, 1):
                        if hh == 0:
                            ps_h[0] = pp.tile([128, 512], F32, tag="pp",
                                              bufs=2, name=f"pj{sc_i}_0")
                            ps_h[1] = pp.tile([128, 512], F32, tag="pp",
                                              bufs=2, name=f"pj{sc_i}_1")
                            for j4 in range(0, kc_n, 4):
                                for g in (emit_group(0, ps_h[0], range(j4, j4 + 4)),
                                          emit_group(1, ps_h[1], range(j4, j4 + 4))):
                                    yield from g
                        ps = ps_h[hh]
                    else:
                        ps = pp.tile([128, 512], F32, tag="pp", bufs=2,
                                     name=f"pj{sc_i}_{hh}")
                        yield from emit_group(hh, ps, range(kc_n))
                    if hh < NH:
                        qtr[(hh, sc_i)] = apool.tile(
                            [128, 512], BF16, tag=f"qtr{hh}", bufs=2,
                            name=f"qtr{hh}_{sc_i}")
                        rope(qtr[(hh, sc_i)][:], ps, sc_i)
                    elif hh == NH:
                        rope(ktr[:, sc_i * 512:(sc_i + 1) * 512], ps, sc_i)
                    else:
                        # vT [dh, 512] -> copy to SBUF -> xbar-transpose to
                        # natural [s, dh] 128-blocks
                        vt_sb = tpool.tile([128, 512], BF16, tag="vt", bufs=2,
                                           name=f"vt{sc_i}")
                        nc.scalar.copy(vt_sb[:], ps[:])
                        for stl in range(4):
                            st = sc_i * 4 + stl
                            v_sb[st] = apool.tile([128, DH], BF16,
                                                  tag=f"v{st}", name=f"v{st}")
                            nc.sync.dma_start_transpose(
                                v_sb[st][:],
                                vt_sb[:, stl * 128:(stl + 1) * 128])

            def gen_outproj(qb, alt_psum=False, mix_copies=False):
                """Output projection for q-block qb; yields after each PE op.

                When run as filler inside the attention loop, the osb
                PSUM-evacuation copies must stay OFF the vector engine
                (it is busy with l_acc chains and would stall the wop
                PSUM rotation); ACT has plenty of idle there.
                """
                for stl in range(4):
                    st = 4 * qb + stl
                    for dm in range(4):
                        if alt_psum and (stl * 4 + dm) % 2 == 1:
                            wopt = pp.tile([128, 2, 512], F32, tag="sc",
                                           bufs=2, name=f"wop{st}_{dm}")
                            wop = wopt[:, 0, :]
                        else:
                            wop = pp.tile([128, 512], F32, tag="pp", bufs=2,
                                          name=f"wop{st}_{dm}")[:]
                        for h in range(NH):
                            nc.tensor.matmul(
                                wop,
                                otr[(h, qb)][:, stl * 128:(stl + 1) * 128],
                                wo_sb[:, h, dm * 512:(dm + 1) * 512],
                                start=(h == 0), stop=(h == NH - 1))
                            yield
                        osb = tpool.tile([128, 512], BF16, tag="osb", bufs=4,
                                         name=f"osb{st}_{dm}")
                        if mix_copies and (st + dm) % 2 == 0:
                            nc.vector.tensor_copy(osb[:], wop)
                        else:
                            nc.scalar.copy(osb[:], wop)
                        nc.sync.dma_start(
                            out_p[st * 128:(st + 1) * 128,
                                  dm * 512:(dm + 1) * 512], osb[:])

            fillers = []

            def pull(n):
                for _ in range(n):
                    while fillers:
                        try:
                            next(fillers[0])
                            break
                        except StopIteration:
                            fillers.pop(0)
                    else:
                        return

            def drain(gen):
                for _ in gen:
                    pass

            def attn(qb):
                """Attention for q-block qb (4 heads), pulling PE fillers."""
                npair = 2 * (qb + 1)
                pn = 2
                deferred = [None]

                def flush_deferred():
                    if deferred[0] is not None:
                        deferred[0]()
                        deferred[0] = None

                for h in range(NH):
                    q_t = qtr[(h, qb)]
                    l_acc = tpool.tile([128, 512], BF16, tag="lacc", bufs=2,
                                       name=f"lacc{h}_{qb}")
                    otp = pp.tile([128, 512], F32, tag="ot", bufs=2,
                                  name=f"otp{h}_{qb}")
                    pts = []
                    # scores + exp + mask + denominator accumulate
                    for j in range(npair):
                        jd = j - 2 * qb  # >=0 on diagonal pairs
                        qo = 256 * jd if jd > 0 else 0
                        scp = pp.tile([128, 2, 512], F32, tag="sc", bufs=2,
                                      name=f"scp{h}_{qb}_{j}")
                        nc.tensor.matmul(scp[:, 0, qo:],
                                         ktr[:, (2 * j) * 128:(2 * j + 1) * 128],
                                         q_t[:, qo:], start=True, stop=True)
                        pull(pn)
                        nc.tensor.matmul(scp[:, 1, qo:],
                                         ktr[:, (2 * j + 1) * 128:(2 * j + 2) * 128],
                                         q_t[:, qo:], start=True, stop=True)
                        pull(pn)
                        pt = tpool.tile([128, 2, 512], BF16, tag="pt", bufs=12,
                                        name=f"pt{h}_{qb}_{j}")
                        pts.append((pt, qo))
                        nc.scalar.activation(
                            pt[:, :, qo:], scp[:, :, qo:],
                            mybir.ActivationFunctionType.Exp,
                            scale=float(INV_SQRT_DH))
                        if jd >= 0:
                            # chunk 2j: triangle in cols [qo, qo+128)
                            nc.vector.tensor_mul(pt[:, 0, qo:qo + 128],
                                                 pt[:, 0, qo:qo + 128], mA[:])
                            # chunk 2j+1: first 128 cols dead + triangle
                            nc.vector.tensor_mul(pt[:, 1, qo:qo + 256],
                                                 pt[:, 1, qo:qo + 256], mB[:])
                        if j == 0:
                            nc.vector.tensor_copy(l_acc[:], pt[:, 0, :])
                        else:
                            nc.vector.tensor_add(l_acc[:, qo:], l_acc[:, qo:],
                                                 pt[:, 0, qo:])
                        nc.vector.tensor_add(l_acc[:, qo:], l_acc[:, qo:],
                                             pt[:, 1, qo:])
                    # start the denominator reduce now -- l_acc is complete
                    # after the scores loop, so it overlaps the P@V matmuls
                    otr[(h, qb)] = apool.tile([128, 512], BF16, tag=f"otr{h}",
                                              bufs=2, name=f"otr{h}_{qb}")
                    last = qb == qb_n - 1 and h == NH - 1
                    if last:
                        # last head of the kernel: shortest-latency tail so
                        # the final output projection isn't held up
                        lpt = pp.tile([128, 2, 512], F32, tag="sc", bufs=2,
                                      name=f"lp{h}_{qb}")
                        nc.tensor.matmul(lpt[0:1, 0, :], ones_col[:],
                                         l_acc[:], start=True, stop=True)
                        rlb = tpool.tile([128, 512], F32, tag="rlbs", bufs=2,
                                         name=f"rlbs{h}_{qb}")
                        rl = tpool.tile([1, 512], F32, tag="rl", bufs=2,
                                        name=f"rl{h}_{qb}")
                        nc.vector.reciprocal_approx_fast(rl[:], lpt[0:1, 0, :])
                        nc.gpsimd.partition_broadcast(rlb[:], rl[:])
                    else:
                        lred = tpool.tile([128, 512], F32, tag="lred", bufs=2,
                                          name=f"lred{h}_{qb}")
                        nc.gpsimd.partition_all_reduce(
                            lred[:], l_acc[:], 128, bass.bass_isa.ReduceOp.add)
                    # previous head's normalize now (its reduce is done,
                    # so these DVE ops don't stall the vector FIFO)
                    flush_deferred()
                    # P @ V (dense on PE)
                    for j in range(npair):
                        pt, qo = pts[j]
                        nc.tensor.matmul(otp[:, qo:], v_sb[2 * j][:],
                                         pt[:, 0, qo:], start=(j == 0),
                                         stop=False)
                        nc.tensor.matmul(otp[:, qo:], v_sb[2 * j + 1][:],
                                         pt[:, 1, qo:], start=False,
                                         stop=(j == npair - 1))
                    if last:
                        nc.vector.tensor_mul(otr[(h, qb)][:], otp[:], rlb[:])
                    else:
                        def tail(h=h, lred=lred, otp=otp):
                            rlb2 = tpool.tile([128, 512], F32, tag="rlbs",
                                              bufs=2, name=f"rlbs{h}_{qb}")
                            nc.vector.reciprocal_approx_fast(rlb2[:], lred[:])
                            nc.vector.tensor_mul(otr[(h, qb)][:], otp[:],
                                                 rlb2[:])

                        deferred[0] = tail
                    pull(2)
                flush_deferred()

            # ---- main schedule ----
            drain(gen_proj(0))
            for qb in range(qb_n):
                if qb + 1 < qb_n:
                    g = gen_proj(qb + 1)
                    next(g, None)  # prime: emit DMAs early
                    # proj first in the pull order: outproj fillers bank up
                    # across q-blocks so attn(3) has enough PE filler work
                    fillers.insert(0, g)
                    attn(qb)
                    # proj(qb+1) must fully drain before attn(qb+1): its
                    # rope outputs feed the next q-block's scores.  (If g
                    # is already exhausted this is a no-op; pull() will
                    # pop the spent generator from the list later.)
                    drain(g)
                    fillers.append(gen_outproj(qb))
                else:
                    attn(qb)
            while fillers:
                drain(fillers.pop(0))
            drain(gen_outproj(qb_n - 1, alt_psum=True, mix_copies=True))

    nc.compile()
    return nc


_PROGRAM = None


def _get_program():
    global _PROGRAM
    if _PROGRAM is None:
        _PROGRAM = build_program()
    return _PROGRAM


_DEINT = np.concatenate([np.arange(0, DH, 2), np.arange(1, DH, 2)])


def make_in_maps(x, rope_cos, rope_sin, Wq, Wk, Wv, Wo, s=S):
    cosT = rope_cos[:s].T.astype(np.float32)   # [64, s]
    sinT = rope_sin[:s].T.astype(np.float32)
    cosD = np.concatenate([cosT, cosT], axis=0).astype(ml_dtypes.bfloat16)
    sinS = np.concatenate([sinT, -sinT], axis=0).astype(ml_dtypes.bfloat16)
    p = np.arange(128)[:, None]
    maskA = (np.arange(128)[None, :] >= p).astype(ml_dtypes.bfloat16)
    maskB = (np.arange(256)[None, :] >= p + 128).astype(ml_dtypes.bfloat16)
    in_maps = []
    for c in range(N_CORES):
        b, g = divmod(c, 4)
        xTc = np.ascontiguousarray(x[b].T.astype(ml_dtypes.bfloat16))
        wq_cols = [
            Wq[:, (g * NH + j) * DH:(g * NH + j + 1) * DH][:, _DEINT]
            for j in range(NH)
        ]
        wq_c = np.ascontiguousarray(
            np.concatenate(wq_cols, axis=1).astype(ml_dtypes.bfloat16))
        wk_c = Wk[:, g * DH:(g + 1) * DH][:, _DEINT]
        wv_c = Wv[:, g * DH:(g + 1) * DH]
        wkv_c = np.ascontiguousarray(
            np.concatenate([wk_c, wv_c], axis=1).astype(ml_dtypes.bfloat16))
        wo_c = np.ascontiguousarray(
            Wo[g * NH * DH:(g + 1) * NH * DH, :].astype(ml_dtypes.bfloat16))
        in_maps.append({
            "xT": xTc, "wq": wq_c, "wkv": wkv_c, "wo": wo_c,
            "cosD": np.ascontiguousarray(cosD),
            "sinS": np.ascontiguousarray(sinS),
            "maskA": maskA, "maskB": maskB,
        })
    return in_maps


def kernel(x, rope_cos, rope_sin, Wq, Wk, Wv, Wo):
    nc = _get_program()
    in_maps = make_in_maps(x, rope_cos, rope_sin, Wq, Wk, Wv, Wo)
    res = run_bass_kernel_spmd(nc, in_maps, list(range(N_CORES)))
    out = np.zeros((B, S, D), dtype=np.float32)
    for c in range(N_CORES):
        b, g = divmod(c, 4)
        out[b] += np.asarray(res.results[c]["out_p"]).astype(np.float32)
    return out
